# revision 1
# baseline (speedup 1.0000x reference)
"""Bass/Trainium2 kernel for the BayesianVectorRenderer problem.

Renders a closed cubic-Bezier path into a [1024,1024,4] RGBA image via a
soft winding-number accumulation.

Strategy (8 NeuronCores, SPMD):
  - Host: sample the Bezier path (512 points -> 512 edges).  The image is
    split into 64-row blocks; each core gets 2 blocks, greedily packed to
    balance work.  An edge is active for a core only where its validity
    window (t in ~[-0.6,1.6]) overlaps the core's rows (~110 of 512).
  - Decomposition (winding = FS - sum_j w_j*sigmoid(c - xc_j)):
      FS     = sum of all edge weights (winding at the far left),
      window = per edge, sigmoid(c - xc) differs from its limits only on
               [xcmin-16, xcmax+16]; each core sorts its edges by window
               center so the j-th slot's union window across cores is a
               TIGHT STATIC column range [S_j, B_j) (j-th order statistics
               of similar distributions align),
      step   = w * H(c - B_j) with B_j the slot's static 64-px boundary;
               steps sharing a boundary are mask-summed per partition once
               (tensor_tensor_reduce) and applied as ONE tail-add per
               bucket -- O(16) ops instead of O(edges).
  - ScalarEngine evaluates the sigmoid windows (per-partition bias = -xc),
    VectorEngine does fused multiply-accumulate (scalar_tensor_tensor),
    bucket reduces and tail adds.  Raw Bass with explicit semaphores
    (this walrus build rejects Tile's embedded on_wait encoding).
"""

from contextlib import ExitStack

import numpy as np

import concourse.bass as bass
from concourse import mybir
from concourse.bass_utils import run_bass_kernel_spmd

H = 1024
W = 1024
SAMPLES_PER_SEG = 32
N_CORES = 8
ROWS = H // N_CORES
BS = 64              # row-block size for load balancing
BKT = 64             # column bucket size for shared step-adds

T_LO = np.float32(-0.6)
T_HI = np.float32(1.6)
MARGIN = 16          # sigmoid(16) is within 1.2e-7 of 1

N_ACC = 2   # accumulators (breaks the DVE in-place RAW chain)
N_SIG = 16  # sigmoid buffer depth (ACT runs ahead of DVE)


def _sample_bezier(cp: np.ndarray) -> np.ndarray:
    """Faithful fp32 port of reference.sample_bezier_path."""
    cp = cp.astype(np.float32)
    n = cp.shape[0]
    s = (n - 1) // 3
    idx = 3 * np.arange(s)
    p0 = cp[idx][:, None, :]
    p1 = cp[idx + 1][:, None, :]
    p2 = cp[idx + 2][:, None, :]
    p3 = cp[idx + 3][:, None, :]
    t = np.linspace(0.0, 1.0, SAMPLES_PER_SEG, dtype=np.float32)[None, :, None]
    mt = (np.float32(1.0) - t).astype(np.float32)
    pts = (
        (mt * mt * mt) * p0
        + np.float32(3.0) * (mt * mt) * t * p1
        + np.float32(3.0) * mt * (t * t) * p2
        + (t * t * t) * p3
    )
    return pts.reshape(s * SAMPLES_PER_SEG, 2).astype(np.float32)


def _build_nc(starts, ends, buckets):
    """Build the SPMD Bass graph.

    Slot j's window is the static column range [starts[j], ends[j]);
    ends[j] is also the slot's step boundary (64-px aligned).  Slots are
    ordered so same-boundary slots are contiguous: buckets is a tuple of
    (B, slot_lo, slot_hi) ranges, each getting one slice-reduce and one
    shared tail-add over [B, W).
    Packed input [ROWS, 5e+4]: [inv | b | dx | x0 | cf | y | color]
    """
    e = len(starts)
    nb = len(buckets)
    wmax = max(en - st for st, en in zip(starts, ends))
    tot = 5 * e + 1 + 3
    nc = bass.Bass("TRN2", target_bir_lowering=False, debug=False)
    f32 = mybir.dt.float32
    A = mybir.AluOpType
    SIG = mybir.ActivationFunctionType.Sigmoid

    d_inp = nc.declare_dram_parameter("inp", [ROWS, tot], f32, isOutput=False)
    d_out = nc.declare_dram_parameter("out", [ROWS, W * 4], f32, isOutput=True)

    with ExitStack() as ctx:
        t_inp = ctx.enter_context(nc.sbuf_tensor([ROWS, tot], f32))
        t_b0 = ctx.enter_context(nc.sbuf_tensor([ROWS, 1], f32))
        t_b20 = ctx.enter_context(nc.sbuf_tensor([ROWS, 1], f32))
        t_fs = ctx.enter_context(nc.sbuf_tensor([ROWS, 1], f32))
        t_bs = ctx.enter_context(nc.sbuf_tensor([ROWS, max(nb, 1)], f32))
        t_grid = ctx.enter_context(nc.sbuf_tensor([ROWS, W], f32))
        t_t = ctx.enter_context(nc.sbuf_tensor([ROWS, e], f32))
        t_v1 = ctx.enter_context(nc.sbuf_tensor([ROWS, e], f32))
        t_v2 = ctx.enter_context(nc.sbuf_tensor([ROWS, e], f32))
        t_w = ctx.enter_context(nc.sbuf_tensor([ROWS, e], f32))
        t_wn = ctx.enter_context(nc.sbuf_tensor([ROWS, e], f32))
        t_xc = ctx.enter_context(nc.sbuf_tensor([ROWS, e], f32))
        t_nxc = ctx.enter_context(nc.sbuf_tensor([ROWS, e], f32))
        t_scr = ctx.enter_context(nc.sbuf_tensor([ROWS, e], f32))
        t_sig = ctx.enter_context(nc.sbuf_tensor([ROWS, N_SIG * wmax], f32))
        t_acc = ctx.enter_context(nc.sbuf_tensor([ROWS, N_ACC * W], f32))
        t_alpha = ctx.enter_context(nc.sbuf_tensor([ROWS, W], f32))
        t_rgba = ctx.enter_context(nc.sbuf_tensor([ROWS, W * 4], f32))
        dma_in = ctx.enter_context(nc.semaphore("dma_in"))
        pool_sem = ctx.enter_context(nc.semaphore("pool_sem"))
        dve_p1 = ctx.enter_context(nc.semaphore("dve_p1"))
        act_p = ctx.enter_context(nc.semaphore("act_p"))
        dve_p2 = ctx.enter_context(nc.semaphore("dve_p2"))
        act_loop = ctx.enter_context(nc.semaphore("act_loop"))
        dve_loop = ctx.enter_context(nc.semaphore("dve_loop"))
        act_alpha = ctx.enter_context(nc.semaphore("act_alpha"))
        dve_fin = ctx.enter_context(nc.semaphore("dve_fin"))
        dve_scr = ctx.enter_context(nc.semaphore("dve_scr"))
        dma_out = ctx.enter_context(nc.semaphore("dma_out"))
        block = ctx.enter_context(nc.Block())

        inp = t_inp[:]
        s_inv = inp[:, 0:e]
        s_b = inp[:, e:2 * e]
        s_dx = inp[:, 2 * e:3 * e]
        s_x0 = inp[:, 3 * e:4 * e]
        s_cf = inp[:, 4 * e:5 * e]
        s_y = inp[:, 5 * e:5 * e + 1]
        s_col = inp[:, 5 * e + 1:5 * e + 4]
        sig = [t_sig[:][:, k * wmax:(k + 1) * wmax] for k in range(N_SIG)]
        acc = [t_acc[:][:, k * W:(k + 1) * W] for k in range(N_ACC)]
        rgba4 = t_rgba[:].rearrange("p (c k) -> p c k", k=4)

        # dve_loop counting: 1 (acc0 init) + nb (tails) + e (loop) + 3 (folds)
        n_pre = 1 + nb
        n_loop = n_pre + e

        @block.sync
        def _(sync):
            sync.dma_start(out=inp, in_=d_inp[:]).then_inc(dma_in, 16)
            sync.wait_ge(dve_fin, 1)
            sync.wait_ge(pool_sem, 3)
            sync.dma_start(out=d_out[:], in_=t_rgba[:]).then_inc(dma_out, 16)

        @block.gpsimd
        def _(gpsimd):
            gpsimd.iota(
                t_grid[:], pattern=[[1, W]], base=0, channel_multiplier=0,
                allow_small_or_imprecise_dtypes=True,
            ).then_inc(pool_sem, 1)
            for k in range(1, N_ACC):
                gpsimd.memset(acc[k], 0.0)
            gpsimd.memset(acc[0], 0.0).then_inc(pool_sem, 1)
            # constant color channels of the output (Pool is otherwise idle)
            gpsimd.wait_ge(dma_in, 16)
            gpsimd.wait_ge(pool_sem, 1)  # own iota completed (t_grid RAW)
            for ch in range(2):
                gpsimd.tensor_scalar(
                    rgba4[:, :, ch], t_grid[:], 0.0, s_col[:, ch:ch + 1],
                    op0=A.mult, op1=A.add,
                )
            gpsimd.tensor_scalar(
                rgba4[:, :, 2], t_grid[:], 0.0, s_col[:, 2:3],
                op0=A.mult, op1=A.add,
            ).then_inc(pool_sem, 1)

        @block.vector
        def _(vector):
            vector.memset(t_b0[:], 0.0)
            vector.memset(t_b20[:], 20.0)
            vector.wait_ge(dma_in, 16)
            vector.scalar_tensor_tensor(
                t_t[:], s_inv, s_y, s_b, op0=A.mult, op1=A.subtract,
            ).then_inc(dve_p1, 1)
            vector.wait_ge(act_p, 1)
            vector.tensor_tensor(t_w[:], t_v1[:], t_v2[:], A.mult).then_inc(
                dve_scr, 1)
            vector.tensor_tensor(t_xc[:], t_t[:], s_dx, A.mult).then_inc(
                dve_scr, 1)
            vector.wait_ge(dve_scr, 2)
            vector.tensor_tensor(t_w[:], t_w[:], s_cf, A.mult).then_inc(
                dve_scr, 1)
            vector.tensor_tensor(t_xc[:], t_xc[:], s_x0, A.add).then_inc(
                dve_scr, 1)
            vector.wait_ge(dve_scr, 4)
            vector.tensor_scalar(t_wn[:], t_w[:], -1.0, None, op0=A.mult
                                 ).then_inc(dve_scr, 1)
            vector.tensor_scalar(
                t_nxc[:], t_xc[:], -1.0, None, op0=A.mult).then_inc(dve_p2, 1)
            vector.wait_ge(dve_scr, 5)
            # FS = sum of weights; per-bucket sums of -w
            vector.tensor_scalar(
                t_v1[:], t_w[:], 1.0, 0.0, op0=A.mult, op1=A.add,
                accum_out=t_fs[:, 0:1],
            ).then_inc(dve_scr, 1)
            for b, (bb_col, slo, shi) in enumerate(buckets):
                # disjoint t_scr/t_bs regions: no WAW chain needed
                vector.tensor_scalar(
                    t_scr[:, slo:shi], t_wn[:, slo:shi], 1.0, 0.0,
                    op0=A.mult, op1=A.add, accum_out=t_bs[:, b:b + 1],
                ).then_inc(dve_scr, 1)
            if nb == 0:
                vector.tensor_scalar(
                    t_bs[:, 0:1], t_fs[:, 0:1], 0.0, None, op0=A.mult,
                ).then_inc(dve_scr, 1)
            vector.wait_ge(dve_scr, 6 + max(nb, 1))
            vector.wait_ge(pool_sem, 2)
            # acc0 = FS
            vector.tensor_scalar(
                acc[0], acc[0], 0.0, t_fs[:, 0:1], op0=A.mult, op1=A.add,
            ).then_inc(dve_loop, 1)
            # shared step tail-adds (commutative: run BEFORE the window loop,
            # filling the DVE idle gap while ACT produces the first sigmoids)
            for b, (bb_col, slo, shi) in enumerate(buckets):
                vector.wait_ge(dve_loop, max(1, 1 + b - N_ACC + 1))
                vector.tensor_scalar(
                    acc[b % N_ACC][:, bb_col:W], acc[b % N_ACC][:, bb_col:W],
                    t_bs[:, b:b + 1], None, op0=A.add,
                ).then_inc(dve_loop, 1)
            for j in range(e):
                st, en = starts[j], ends[j]
                vector.wait_ge(act_loop, j + 1)
                vector.wait_ge(dve_loop, n_pre if j < N_ACC else n_pre + j - N_ACC + 1)
                a = acc[j % N_ACC]
                # acc[:, st:en] += sig * (-w_j)
                vector.scalar_tensor_tensor(
                    a[:, st:en], sig[j % N_SIG][:, 0:en - st],
                    t_wn[:, j:j + 1], a[:, st:en], op0=A.mult, op1=A.add,
                ).then_inc(dve_loop, 1)
            vector.wait_ge(dve_loop, n_loop)
            vector.tensor_tensor(acc[0], acc[0], acc[1], A.add).then_inc(dve_loop, 1)
            vector.wait_ge(act_alpha, 1)
            vector.tensor_copy(rgba4[:, :, 3], t_alpha[:]).then_inc(dve_fin, 1)

        @block.scalar
        def _(scalar):
            scalar.wait_ge(dve_p1, 1)
            scalar.activation(t_v1[:], t_t[:], SIG, bias=t_b0[:, 0:1], scale=20.0)
            scalar.activation(
                t_v2[:], t_t[:], SIG, bias=t_b20[:, 0:1], scale=-20.0,
            ).then_inc(act_p, 1)
            scalar.wait_ge(dve_p2, 1)
            scalar.wait_ge(pool_sem, 1)
            for j in range(e):
                st, en = starts[j], ends[j]
                if j >= N_SIG:
                    scalar.wait_ge(dve_loop, n_pre + j - N_SIG + 1)
                # sig = sigmoid(c - xc_j) over the slot's static window
                scalar.activation(
                    sig[j % N_SIG][:, 0:en - st], t_grid[:, st:en], SIG,
                    bias=t_nxc[:, j:j + 1], scale=1.0,
                ).then_inc(act_loop, 1)
            scalar.wait_ge(dve_loop, n_loop + 1)
            scalar.activation(
                t_alpha[:], acc[0], SIG, bias=t_b0[:, 0:1], scale=4.0,
            ).then_inc(act_alpha, 1)

    return nc


def _xc_at(x0, y0, inv, dx, y):
    return x0 + (y - y0) * inv * dx


def _prepare(control_points: np.ndarray, color: np.ndarray):
    """Host prep: plan the decomposition, build the graph + input maps."""
    cp = np.asarray(control_points, dtype=np.float32)
    col = np.asarray(color, dtype=np.float32)

    pts = _sample_bezier(cp)
    nxt = np.roll(pts, -1, axis=0)
    x0 = pts[:, 0]
    y0 = pts[:, 1]
    dy = nxt[:, 1] - y0
    dx = nxt[:, 0] - x0
    coeff = (np.sign(dy) * (np.abs(dy) >= np.float32(1e-6))).astype(np.float32)
    inv = (np.float32(1.0) / (dy + np.float32(1e-8))).astype(np.float32)
    b_arr = (y0 * inv).astype(np.float32)

    ya = y0 + T_LO * dy
    yb = y0 + T_HI * dy
    ymin = np.minimum(ya, yb)
    ymax = np.maximum(ya, yb)

    n_blocks = H // BS
    per_core = ROWS // BS
    blk_sets = []
    for b in range(n_blocks):
        r0 = b * BS
        s = np.nonzero((coeff != 0) & (ymax >= r0) & (ymin <= r0 + BS - 1))[0]
        blk_sets.append(set(s.tolist()))

    def window(j, blocks):
        """Static column window [lo, hi) of edge j over the blocks' rows."""
        xmn = xmx = None
        for b in blocks:
            r0, r1 = b * BS, b * BS + BS - 1
            a = max(ymin[j], r0)
            bb = min(ymax[j], r1)
            if a > bb:
                continue
            v0 = _xc_at(x0[j], y0[j], inv[j], dx[j], a)
            v1 = _xc_at(x0[j], y0[j], inv[j], dx[j], bb)
            lo, hi = min(v0, v1), max(v0, v1)
            xmn = lo if xmn is None else min(xmn, lo)
            xmx = hi if xmx is None else max(xmx, hi)
        if xmn is None:
            return None
        if not (np.isfinite(xmn) and np.isfinite(xmx)):
            return (0, W)
        if xmx + MARGIN <= 0:
            return None  # fully left of canvas: contributes ~0
        lo = int(np.clip(np.floor(xmn) - MARGIN, 0, W - 1))
        hi = int(np.clip(np.ceil(xmx) + MARGIN, lo + 1, W))
        return (lo, hi)

    # greedy pack 2 blocks per core, minimizing estimated engine time
    order = sorted(range(n_blocks), key=lambda i: -len(blk_sets[i]))
    core_edge_sets = [set() for _ in range(N_CORES)]
    core_blocks = [[] for _ in range(N_CORES)]
    for i in order:
        best, best_cost = None, None
        for c in range(N_CORES):
            if len(core_blocks[c]) >= per_core:
                continue
            u = core_edge_sets[c] | blk_sets[i]
            cost = 0
            for j in u:
                g = window(j, core_blocks[c] + [i])
                if g is not None:
                    cost += 300 + (g[1] - g[0])
            if best_cost is None or cost < best_cost:
                best_cost, best = cost, c
        core_edge_sets[best] |= blk_sets[i]
        core_blocks[best].append(i)

    # per-core edge windows, sorted by window center
    core_lists = []
    for c in range(N_CORES):
        lst = []
        for j in sorted(core_edge_sets[c]):
            g = window(j, core_blocks[c])
            if g is not None:
                lst.append((j, g[0], g[1]))
        lst.sort(key=lambda t: t[1] + t[2])
        core_lists.append(lst)

    e = max(8, int(np.ceil(max(len(l) for l in core_lists) / 8.0)) * 8)
    starts = [W] * e
    ends = [0] * e
    for c in range(N_CORES):
        core_lists[c] = core_lists[c][:e]
        for s, (j, lo, hi) in enumerate(core_lists[c]):
            starts[s] = min(starts[s], lo)
            ends[s] = max(ends[s], hi)
    # slot boundary: 64-px aligned end (also the step start)
    for s in range(e):
        if ends[s] == 0:  # pure padding slot, no real edge in any core
            starts[s], ends[s] = W - 8, W
            continue
        ends[s] = int(min(W, int(np.ceil(ends[s] / BKT)) * BKT))
        starts[s] = min(starts[s], ends[s] - 8)

    # permute slots so same-boundary slots are contiguous (bucket slices)
    perm = sorted(range(e), key=lambda s: (ends[s], s))
    starts = [starts[s] for s in perm]
    ends = [ends[s] for s in perm]
    inv_lists = []
    for c in range(N_CORES):
        old = core_lists[c]
        inv_lists.append([old[s] if s < len(old) else None for s in perm])

    buckets = []
    s = 0
    while s < e:
        en = ends[s]
        s2 = s
        while s2 < e and ends[s2] == en:
            s2 += 1
        if en < W:
            buckets.append((en, s, s2))
        s = s2

    nc = _build_nc(tuple(starts), tuple(ends), tuple(buckets))

    in_maps = []
    core_rows = []
    for c in range(N_CORES):
        lst = inv_lists[c]

        def gather(a):
            g = np.zeros(e, np.float32)
            for s, t in enumerate(lst):
                if t is not None:
                    g[s] = a[t[0]]
            return g[None, :]

        rows = np.concatenate(
            [np.arange(b * BS, (b + 1) * BS) for b in sorted(core_blocks[c])]
        )
        core_rows.append(rows)
        y_vec = rows.astype(np.float32)[:, None]
        segs = [gather(inv), gather(b_arr), gather(dx), gather(x0),
                gather(coeff)]
        segs += [np.zeros((1, 1), np.float32), col[None, :]]
        row = np.concatenate(segs, axis=1)
        packed = np.broadcast_to(row, (ROWS, row.shape[1])).copy()
        packed[:, 5 * e:5 * e + 1] = y_vec
        in_maps.append({"inp": packed})

    return nc, in_maps, core_rows


def kernel(control_points: np.ndarray, color: np.ndarray) -> np.ndarray:
    nc, in_maps, core_rows = _prepare(control_points, color)
    results = run_bass_kernel_spmd(nc, in_maps, core_ids=list(range(N_CORES))).results
    out = np.empty((H, W, 4), dtype=np.float32)
    for c in range(N_CORES):
        out[core_rows[c]] = results[c]["out"].reshape(ROWS, W, 4)
    return out



# revision 3
# speedup vs baseline: 1.6498x; 1.6498x over previous
"""Bass/Trainium2 kernel for the BayesianVectorRenderer problem.

Renders a closed cubic-Bezier path into a [1024,1024,4] RGBA image via a
soft winding-number accumulation.

Band-scheme decomposition (8 NeuronCores, SPMD, 128 rows/core):

  winding(y,x) = FS(y) - cumsum_x D(y,x)
  D(y,x)       = sum_j w_j(y) * [sig(x - xc_j(y)) - sig(x-1 - xc_j(y))]

Each edge's sigmoid transition is evaluated only inside a narrow window
[xc_min - M, xc_max + M] placed on a lattice of fixed-width bands
("slots").  Edges with disjoint active rows share a slot (per-partition
xcr/w data).  All heavy passes are a handful of giant instructions:

  arg  = ramp - xcr_bcast        one TT per class chunk   (DVE/Pool, f32)
  sgm  = sigmoid(arg)            one ACT per class chunk  (f16 out)
  d    = sgm[i] - sgm[i-1]       one TT per class chunk   (f16, gap cols)
  dw   = d * wneg_bcast          one TT per class chunk   (f16)
  D   += dw at band positions    phase-segment TTs        (f16, ping-pong)
  wind = FS + cumsum(D)          tensor_tensor_scan
  alpha= sigmoid(4*wind)         ACT, f16 -> DMA out; host composes RGBA

Host does O(edges x rows) scalar prep (xc, w, FS, slot packing); the
device does all per-pixel work.
"""

from contextlib import ExitStack

import numpy as np

import concourse.bass as bass
from concourse import mybir
from concourse.bass_utils import run_bass_kernel_spmd

H = 1024
W = 1024
SAMPLES_PER_SEG = 32
N_CORES = 8
ROWS = 128
BS = 64
PADL = 32
PADR = 32
DWID = PADL + W + PADR
M = 11
T_LO = np.float32(-0.35)
T_HI = np.float32(1.35)
XC_LO = -20.0
XC_HI = W + 21.0

CLASSES = [(160, 80), (320, 160), (DWID, DWID)]


def _sample_bezier(cp):
    cp = cp.astype(np.float32)
    n = cp.shape[0]
    s = (n - 1) // 3
    idx = 3 * np.arange(s)
    p0 = cp[idx][:, None, :]
    p1 = cp[idx + 1][:, None, :]
    p2 = cp[idx + 2][:, None, :]
    p3 = cp[idx + 3][:, None, :]
    t = np.linspace(0.0, 1.0, SAMPLES_PER_SEG, dtype=np.float32)[None, :, None]
    mt = (np.float32(1.0) - t).astype(np.float32)
    pts = (
        (mt * mt * mt) * p0
        + np.float32(3.0) * (mt * mt) * t * p1
        + np.float32(3.0) * mt * (t * t) * p2
        + (t * t * t) * p3
    )
    return pts.reshape(s * SAMPLES_PER_SEG, 2).astype(np.float32)


def _sig(x):
    return 1.0 / (1.0 + np.exp(-x))


def _plan(control_points):
    cp = np.asarray(control_points, dtype=np.float32)
    pts = _sample_bezier(cp)
    nxt = np.roll(pts, -1, axis=0)
    x0 = pts[:, 0]
    y0 = pts[:, 1]
    dy = nxt[:, 1] - y0
    dx = nxt[:, 0] - x0
    coeff = (np.sign(dy) * (np.abs(dy) >= np.float32(1e-6))).astype(np.float32)
    inv = (np.float32(1.0) / (dy + np.float32(1e-8))).astype(np.float32)

    ya = y0 + T_LO * dy
    yb = y0 + T_HI * dy
    ymin = np.minimum(ya, yb)
    ymax = np.maximum(ya, yb)

    yg = np.arange(H, dtype=np.float32)[:, None]
    t_all = (yg - y0[None, :]) * inv[None, :]
    w_all = _sig(20 * t_all) * _sig(20 * (1 - t_all)) * coeff[None, :]
    xc_all = np.clip(x0[None, :] + t_all * dx[None, :], XC_LO, XC_HI)

    n_blocks = H // BS
    per_core = ROWS // BS
    blk_edges = []
    for b in range(n_blocks):
        r0 = b * BS
        s = np.nonzero((coeff != 0) & (ymax >= r0) & (ymin <= r0 + BS - 1))[0]
        blk_edges.append(s)

    def edge_rows_window(j, mask):
        a = max(ymin[j], 0.0)
        b = min(ymax[j], H - 1.0)
        if a > b:
            return None
        ra, rb = int(np.ceil(a)), int(np.floor(b))
        if ra > rb:
            return None
        rr = np.nonzero(mask[ra:rb + 1])[0]
        if len(rr) == 0:
            return None
        rows = rr + ra
        v = xc_all[rows, j]
        lo = float(np.floor(v.min())) - M
        hi = float(np.ceil(v.max())) + M + 1
        return (lo, hi, rows)

    order = sorted(range(n_blocks), key=lambda i: -len(blk_edges[i]))
    core_blocks = [[] for _ in range(N_CORES)]
    for b in order:
        best, best_cost = None, None
        for c in range(N_CORES):
            if len(core_blocks[c]) >= per_core:
                continue
            blocks = core_blocks[c] + [b]
            mask = np.zeros(H, bool)
            for bb in blocks:
                mask[bb * BS:(bb + 1) * BS] = True
            es = set()
            for bb in blocks:
                es.update(blk_edges[bb].tolist())
            cost = 0.0
            for j in es:
                g = edge_rows_window(j, mask)
                if g is not None:
                    cost += 500.0 + (g[1] - g[0])
            if best_cost is None or cost < best_cost:
                best_cost, best = cost, c
        core_blocks[best].append(b)

    core_rows = []
    core_edges = []
    for c in range(N_CORES):
        blocks = sorted(core_blocks[c])
        rows = np.concatenate([np.arange(b * BS, (b + 1) * BS) for b in blocks])
        core_rows.append(rows)
        mask = np.zeros(H, bool)
        mask[rows] = True
        g2l = -np.ones(H, np.int64)
        g2l[rows] = np.arange(ROWS)
        es = set()
        for b in blocks:
            es.update(blk_edges[b].tolist())
        lst = []
        for j in sorted(es):
            g = edge_rows_window(j, mask)
            if g is not None:
                lo, hi, grows = g
                lst.append((j, lo, hi, g2l[grows]))
        core_edges.append(lst)

    # within-core pre-grouping (row-disjoint, lattice-aware)
    wmain, pmain = CLASSES[0]

    def fits_main(lo, hi):
        if hi - lo > wmain:
            return False
        kmin = int(np.ceil((hi - wmain + PADL) / pmain))
        kmax = int(np.floor((lo + PADL) / pmain))
        return kmax >= max(kmin, 0) and kmin <= (DWID - wmain) // pmain

    core_groups = []
    for c in range(N_CORES):
        groups = []
        for (j, lo, hi, rows) in core_edges[c]:
            bits = 0
            for r in rows:
                bits |= 1 << int(r)
            small = fits_main(lo, hi)
            bestg, bestwid = None, None
            for gi, (glo, ghi, gbits, mem) in enumerate(groups):
                if gbits & bits:
                    continue
                nlo, nhi = min(glo, lo), max(ghi, hi)
                if small and not fits_main(nlo, nhi):
                    continue
                if bestwid is None or (nhi - nlo) < bestwid:
                    bestwid, bestg = nhi - nlo, gi
            if bestg is not None:
                glo, ghi, gbits, mem = groups[bestg]
                groups[bestg] = (min(glo, lo), max(ghi, hi), gbits | bits,
                                 mem + [(j, rows)])
            else:
                groups.append((lo, hi, bits, [(j, rows)]))
        core_groups.append(groups)

    nclass = len(CLASSES)

    def candidates(lo, hi, ci):
        width, pitch = CLASSES[ci]
        if hi - lo > width:
            return []
        if ci == nclass - 1:
            return [0]
        kmin = int(np.ceil((hi - width + PADL) / pitch))
        kmax = int(np.floor((lo + PADL) / pitch))
        lim = (DWID - width) // pitch
        return list(range(max(kmin, 0), min(kmax, lim) + 1))

    occ = {}
    placements = [[] for _ in range(N_CORES)]

    order2 = []
    for c in range(N_CORES):
        for (lo, hi, bits, mem) in core_groups[c]:
            ncand = sum(len(candidates(lo, hi, ci)) for ci in range(nclass))
            order2.append((ncand, c, lo, hi, bits, mem))
    order2.sort(key=lambda t: t[0])

    for (_, c, lo, hi, bits, mem) in order2:
        best = None
        for ci in range(nclass):
            width, pitch = CLASSES[ci]
            for k in candidates(lo, hi, ci):
                sweeps = occ.get((ci, k), [])
                placed = False
                for si, sw in enumerate(sweeps):
                    if sw[c] & bits == 0:
                        cost = ci * 1.0
                        if best is None or cost < best[0]:
                            best = (cost, ci, k, si)
                        placed = True
                        break
                if not placed:
                    cost = width + 50.0 + ci
                    if best is None or cost < best[0]:
                        best = (cost, ci, k, len(sweeps))
            if best is not None and best[0] < 40:
                break
        assert best is not None, f"group ({lo},{hi}) fits nowhere"
        _, ci, k, si = best
        sweeps = occ.setdefault((ci, k), [])
        if si == len(sweeps):
            sweeps.append([0] * N_CORES)
        sweeps[si][c] |= bits
        placements[c].append((mem, ci, k, si))

    # slot layout: class-major, sweep, parity, k ascending; no padding
    slots = []
    slot_index = {}
    class_info = []
    for ci in range(nclass):
        width, pitch = CLASSES[ci]
        nparity = max(1, width // pitch) if ci < nclass - 1 else 1
        ks = sorted(k for (cc, k) in occ if cc == ci)
        start = len(slots)
        segments = []  # (slot_start, n, k0)
        if ks:
            maxsweep = max(len(occ[(ci, k)]) for k in ks)
            for s in range(maxsweep):
                for par in range(nparity):
                    kk = [k for k in ks
                          if len(occ[(ci, k)]) > s and k % nparity == par]
                    run = []
                    for k in kk:
                        if run and k == run[-1] + nparity:
                            run.append(k)
                        else:
                            if run:
                                segments.append((len(slots) - len(run),
                                                 len(run), run[0]))
                            run = [k]
                        slot_index[(ci, k, s)] = len(slots)
                        slots.append((ci, k))
                    if run:
                        segments.append((len(slots) - len(run), len(run),
                                         run[0]))
        class_info.append(dict(start=start, n=len(slots) - start, width=width,
                               pitch=pitch, nparity=nparity,
                               segments=segments))

    e = len(slots)
    band_start = np.zeros(e, np.float32)
    for s, (ci, k) in enumerate(slots):
        band_start[s] = -PADL + k * CLASSES[ci][1]

    xcr_pack, wneg_pack, fs_pack = [], [], []
    for c in range(N_CORES):
        rows = core_rows[c]
        xcr = np.zeros((ROWS, e), np.float32)
        for s, (ci, k) in enumerate(slots):
            xcr[:, s] = CLASSES[ci][0] * 0.5
        wp = np.zeros((ROWS, e), np.float32)
        for (mem, ci, k, si) in placements[c]:
            s = slot_index[(ci, k, si)]
            for (j, lrows) in mem:
                gr = rows[lrows]
                xcr[lrows, s] = xc_all[gr, j] - band_start[s]
                wp[lrows, s] = w_all[gr, j]
        fs = w_all[rows, :].sum(1).astype(np.float32)[:, None]
        xcr_pack.append(np.ascontiguousarray(xcr, np.float32))
        wneg_pack.append(np.ascontiguousarray(-wp, np.float16))
        fs_pack.append(np.ascontiguousarray(fs, np.float32))

    return dict(e=e, slots=slots, class_info=class_info, core_rows=core_rows,
                xcr_pack=xcr_pack, wneg_pack=wneg_pack, fs_pack=fs_pack)


def _build_nc(class_info, e):
    """SPMD Bass graph shared by all cores.

    class_info: per class dict(start, n, width, pitch, nparity, segments);
    only the structure is baked -- xcr/wneg/FS arrive as data.
    """
    f32 = mybir.dt.float32
    f16 = mybir.dt.float16
    A = mybir.AluOpType
    SIG = mybir.ActivationFunctionType.Sigmoid

    live = [ci for ci in class_info if ci["n"] > 0]
    B_arg = sum(ci["n"] * ci["width"] for ci in live)      # arg elems (f32)
    B_sig = sum(ci["n"] * (ci["width"] + 1) for ci in live)  # sig w/ gap col
    rampw = max(ci["width"] for ci in live)

    # buffer offsets per class
    argo, sigo, dwo = {}, {}, {}
    oa = os_ = od = 0
    for ci in live:
        key = ci["start"]
        argo[key], sigo[key], dwo[key] = oa, os_, od
        oa += ci["n"] * ci["width"]
        os_ += ci["n"] * (ci["width"] + 1)
        od += ci["n"] * ci["width"]

    # chunking: split the biggest class into 2 for ACT/DVE overlap
    chunks = []  # (class, slot_lo, slot_hi)
    big = max(range(len(live)), key=lambda i: live[i]["n"] * live[i]["width"])
    for i, ci in enumerate(live):
        if i == big and ci["n"] >= 8:
            h = ci["n"] // 2
            chunks.append((ci, 0, h))
            chunks.append((ci, h, ci["n"]))
        else:
            chunks.append((ci, 0, ci["n"]))

    nc = bass.Bass("TRN2", target_bir_lowering=False, debug=False)
    d_xcr = nc.declare_dram_parameter("xcr", [ROWS, e], f32, isOutput=False)
    d_wn = nc.declare_dram_parameter("wn", [ROWS, e], f16, isOutput=False)
    d_fs = nc.declare_dram_parameter("fs", [ROWS, 1], f32, isOutput=False)
    d_out = nc.declare_dram_parameter("alpha", [ROWS, W], f16, isOutput=True)

    with ExitStack() as ctx:
        t_xcr = ctx.enter_context(nc.sbuf_tensor([ROWS, e], f32))
        t_wn = ctx.enter_context(nc.sbuf_tensor([ROWS, e], f16))
        t_w8 = ctx.enter_context(nc.sbuf_tensor([ROWS, e * 8], f16))
        t_fs = ctx.enter_context(nc.sbuf_tensor([ROWS, 1], f32))
        t_ramp = ctx.enter_context(nc.sbuf_tensor([ROWS, rampw], f32))
        t_arg = ctx.enter_context(nc.sbuf_tensor([ROWS, B_arg], f32))
        t_sig = ctx.enter_context(nc.sbuf_tensor([ROWS, B_sig], f16))
        t_dw = ctx.enter_context(nc.sbuf_tensor([ROWS, B_arg], f16))
        t_D0 = ctx.enter_context(nc.sbuf_tensor([ROWS, DWID], f16))
        t_D1 = ctx.enter_context(nc.sbuf_tensor([ROWS, DWID], f16))
        t_wind = ctx.enter_context(nc.sbuf_tensor([ROWS, DWID], f16))
        t_alpha = ctx.enter_context(nc.sbuf_tensor([ROWS, W], f16))
        dma_in = ctx.enter_context(nc.semaphore("dma_in"))
        pool_s = ctx.enter_context(nc.semaphore("pool_s"))
        sub_v = ctx.enter_context(nc.semaphore("sub_v"))
        sig_s = ctx.enter_context(nc.semaphore("sig_s"))
        dw_s = ctx.enter_context(nc.semaphore("dw_s"))
        fin_s = ctx.enter_context(nc.semaphore("fin_s"))
        block = ctx.enter_context(nc.Block())

        nchunk = len(chunks)

        # per-chunk AP helpers -------------------------------------------
        def aps(ch):
            ci, lo, hi = ch
            wc = ci["width"]
            n = hi - lo
            a0 = argo[ci["start"]] + lo * wc
            s0 = sigo[ci["start"]] + lo * (wc + 1)
            d0 = dwo[ci["start"]] + lo * wc
            arg = bass.AP(t_arg, a0, [[B_arg, ROWS], [wc, n], [1, wc]])
            ramp = bass.AP(t_ramp, 0, [[rampw, ROWS], [0, n], [1, wc]])
            xcr = bass.AP(t_xcr, ci["start"] + lo,
                          [[e, ROWS], [1, n], [0, wc]])
            sig_o = bass.AP(t_sig, s0 + 1, [[B_sig, ROWS], [wc + 1, n], [1, wc]])
            sig_hi = sig_o
            sig_lo = bass.AP(t_sig, s0, [[B_sig, ROWS], [wc + 1, n], [1, wc]])
            dw_o = bass.AP(t_dw, d0, [[B_arg, ROWS], [wc, n], [1, wc]])
            assert wc % 8 == 0
            w8 = bass.AP(t_w8, (ci["start"] + lo) * 8,
                         [[e * 8, ROWS], [8, n], [0, wc // 8], [1, 8]])
            dw4 = bass.AP(t_dw, d0, [[B_arg, ROWS], [wc, n], [8, wc // 8], [1, 8]])
            return dict(arg=arg, ramp=ramp, xcr=xcr, sig_o=sig_o,
                        sig_hi=sig_hi, sig_lo=sig_lo, dw_o=dw_o, w8=w8,
                        dw4=dw4)

        # segment phase list: (seg_idx, class, slot_start, n, k0, chunk_idx)
        seg_list = []
        for ci in live:
            wc, pc, npar = ci["width"], ci["pitch"], ci["nparity"]
            for (s0, n, k0) in ci["segments"]:
                # find covering chunk (segments never straddle the split
                # because the split is at slot h -- verify and fall back)
                local = s0 - ci["start"]
                cidx = None
                for qi, (cj, lo, hi) in enumerate(chunks):
                    if cj is ci and lo <= local and local + n <= hi:
                        cidx = qi
                        break
                seg_list.append((ci, s0, n, k0, cidx))

        @block.sync
        def _(sync):
            sync.dma_start(out=t_xcr[:], in_=d_xcr[:]).then_inc(dma_in, 16)
            sync.dma_start(out=t_wn[:], in_=d_wn[:]).then_inc(dma_in, 16)
            sync.dma_start(out=t_fs[:], in_=d_fs[:]).then_inc(dma_in, 16)
            sync.wait_ge(fin_s, 1)
            sync.dma_start(out=d_out[:], in_=t_alpha[:]).then_inc(dma_in, 16)

        @block.gpsimd
        def _(gpsimd):
            gpsimd.iota(
                t_ramp[:], pattern=[[1, rampw]], base=0, channel_multiplier=0,
                allow_small_or_imprecise_dtypes=True,
            ).then_inc(pool_s, 1)
            gpsimd.memset(t_D0[:], 0.0)
            gpsimd.memset(t_D1[:], 0.0)
            # zero the per-slot gap columns of t_sig
            for ci in live:
                wc, n = ci["width"], ci["n"]
                gap = bass.AP(t_sig, sigo[ci["start"]],
                              [[B_sig, ROWS], [wc + 1, n], [1, 1]])
                gpsimd.memset(gap, 0.0)
            gpsimd.memset(t_sig[:, 0:1], 0.0).then_inc(pool_s, 1)

        @block.vector
        def _(vector):
            vector.wait_ge(dma_in, 32)
            # w8: 8 contiguous copies of each slot weight
            w8o = bass.AP(t_w8, 0, [[e * 8, ROWS], [8, e], [1, 8]])
            w8i = bass.AP(t_wn, 0, [[e, ROWS], [1, e], [0, 8]])
            vector.tensor_copy(w8o, w8i)
            # subtract chunks
            vector.wait_ge(pool_s, 1)
            for q, ch in enumerate(chunks):
                p = aps(ch)
                vector.tensor_tensor(p["arg"], p["ramp"], p["xcr"],
                                     A.subtract).then_inc(sub_v, 1)
            # diff + wmult per chunk (after its sigmoid)
            vector.wait_ge(pool_s, 2)
            for q, ch in enumerate(chunks):
                p = aps(ch)
                vector.wait_ge(sig_s, q + 1)
                vector.tensor_tensor(p["dw_o"], p["sig_hi"], p["sig_lo"],
                                     A.subtract)
                vector.tensor_tensor(p["dw4"], p["dw4"], p["w8"],
                                     A.mult).then_inc(dw_s, 1)
            # phases
            nseg = 0
            for (ci, s0, n, k0, cidx) in seg_list:
                wc, pc, npar = ci["width"], ci["pitch"], ci["nparity"]
                if cidx is not None:
                    vector.wait_ge(dw_s, cidx + 1)
                else:
                    vector.wait_ge(dw_s, nchunk)
                dbuf = t_D0 if (nseg % 2 == 0) else t_D1
                dcol = k0 * pc
                dap = bass.AP(dbuf, dcol,
                              [[DWID, ROWS], [pc * npar, n], [1, wc]])
                sap = bass.AP(t_dw, dwo[ci["start"]] + (s0 - ci["start"]) * wc,
                              [[B_arg, ROWS], [wc, n], [1, wc]])
                vector.tensor_tensor(dap, dap, sap, A.add)
                nseg += 1
            vector.tensor_tensor(t_D0[:], t_D0[:], t_D1[:], A.add)
            vector.wait_ge(dma_in, 48)
            vector.tensor_tensor_scan(
                t_wind[:], t_D0[:], t_D0[:], t_fs[:, 0:1],
                op0=A.add, op1=A.bypass,
            ).then_inc(dw_s, 1)

        @block.scalar
        def _(scalar):
            for q, ch in enumerate(chunks):
                p = aps(ch)
                scalar.wait_ge(sub_v, q + 1)
                if q == 0:
                    scalar.wait_ge(pool_s, 2)
                scalar.activation(p["sig_o"], p["arg"], SIG, bias=0.0,
                                  scale=1.0).then_inc(sig_s, 1)
            scalar.wait_ge(dw_s, nchunk + 1)
            scalar.activation(
                t_alpha[:], t_wind[:, PADL:PADL + W], SIG, bias=0.0,
                scale=4.0,
            ).then_inc(fin_s, 1)

    return nc


def _prepare(control_points, color):
    plan = _plan(control_points)
    nc = _build_nc(plan["class_info"], plan["e"])
    in_maps = []
    for c in range(N_CORES):
        in_maps.append({
            "xcr": plan["xcr_pack"][c],
            "wn": plan["wneg_pack"][c],
            "fs": plan["fs_pack"][c],
        })
    return nc, in_maps, plan["core_rows"]


def kernel(control_points, color):
    col = np.asarray(color, dtype=np.float32)
    nc, in_maps, core_rows = _prepare(control_points, color)
    results = run_bass_kernel_spmd(nc, in_maps,
                                   core_ids=list(range(N_CORES))).results
    out = np.empty((H, W, 4), dtype=np.float32)
    out[:, :, 0] = col[0]
    out[:, :, 1] = col[1]
    out[:, :, 2] = col[2]
    for c in range(N_CORES):
        out[core_rows[c], :, 3] = results[c]["alpha"].astype(np.float32)
    return out
